# revision 55
# baseline (speedup 1.0000x reference)
"""Causal attention kernel for TRN2, 8 NeuronCores (SPMD) — v15.

Math: q = x@Wq.T, kT = Wk@x.T, scores = qk causal-masked, unnormalized
exp softmax, out = (attn@x) @ W2.T.  Sequence-parallel over queries with
stride-8 interleave (core c owns queries {8m+c}) so causal work is
identical on all 8 cores; kT projection replicated (collectives cost
15us fixed + ~40GB/s here — strictly worse than the 27us of replicated
PE work they would save).  fp16 score path, fp16 xv x bf16 at V path
(mixed 16-bit matmul), bf16 W2 path, f32 PSUM accumulation.

Schedule (~187us HW; v10 was ~196us, first version 215us):
  * Single software-pipelined loop over 16 key groups (KG=256): each
    step runs kproj(i), scores(2i-2..2i-1), denom(2i-6..2i-5), and 4 oc
    streams of the previous V group — no instruction waits on the same
    step's DVE/Scalar work.
  * All input DMAs issued on the single sync queue in strict need
    order (DMA engines round-robin across queues, so a second queue
    lets non-critical bytes race ahead of the critical-path feed); xv
    issues run one group ahead of consumption (pool 17-deep).
  * ACT exp table pre-warmed at t=0 (else the first at-tile pays the
    1.28us table load on the critical path).
  * kproj PSUM double-buffered (psk 2) so kproj(i) does not wait on
    the DVE ktile copy of kproj(i-1); V pool at 3 banks to pay for it.
  * Denominator matmuls use a [128,128] all-ones lhs so the sums land
    broadcast across all PSUM partitions (the PE array is 128 wide
    either way) — no gpsimd partition_broadcast; the reciprocal runs
    as four [128,128] DVE chunks interleaved between the final
    v_streams so it never blocks the aoacc merges feeding W2.
    (ACT-engine Reciprocal is blocked by bass for accuracy; bulk ACT
    PSUM-read copies also proved to clock-throttle the whole core.)
  * Softmax normalization folded into the W2 output copies (x recip),
    so W2 matmuls depend only on aoacc; 10 W2 weight loads pre-issued
    at the end of the sync chain (they naturally land ~120-135us).
  * PSUM banks: kproj 2 + scores 2 + denom 1 + V 3.
  * Output in oc pairs ([128,1024] tiles, 2KB DRAM lines); last oc as
    two half-width matmul chains so the final normalize+store overlaps
    the second chain's matmuls.
"""

from contextlib import ExitStack

import numpy as np
import ml_dtypes

import concourse.bass as bass
import concourse.bacc as bacc
import concourse.mybir as mybir
import concourse.tile as tile
from concourse import bass_isa
from concourse.bass_utils import run_bass_kernel_spmd

N_CTX = 4096
D_MODEL = 2048
D_HEAD = 128
NCORES = 8
QPC = N_CTX // NCORES
NKT = N_CTX // 128
NDM = D_MODEL // 128
KG = 256
NKG = N_CTX // KG              # 16
GS = 8
MASK_NEG = -1.0e30

F16 = mybir.dt.float16
BF16 = mybir.dt.bfloat16
F32 = mybir.dt.float32


def _widths():
    return [QPC - 16 * kt for kt in range(NKT)]


def build_program():
    nc = bacc.Bacc(trn_type="TRN2", target_bir_lowering=False, debug=False)

    xqr = nc.declare_dram_parameter("xqr", [128, NDM * QPC], F16, isOutput=False)
    xtp = nc.declare_dram_parameter("xtp", [NKG, 128, NDM * KG], F16, isOutput=False)
    xv = nc.declare_dram_parameter("xv", [N_CTX, D_MODEL], F16, isOutput=False)
    wqr = nc.declare_dram_parameter("wqr", [128, D_MODEL], F16, isOutput=False)
    wkr = nc.declare_dram_parameter("wkr", [128, D_MODEL], F16, isOutput=False)
    w2r = nc.declare_dram_parameter("w2r", [NDM, 128, D_MODEL], BF16, isOutput=False)
    maskb = nc.declare_dram_parameter("maskb", [128, 16], F32, isOutput=False)
    outT2 = nc.declare_dram_parameter("outT2", [128, NDM * QPC], F16, isOutput=True)

    W = _widths()

    with tile.TileContext(nc) as tc:
        with (
            tc.tile_pool(name="static", bufs=1) as st,
            tc.tile_pool(name="xvpool", bufs=17) as xvp,
            tc.tile_pool(name="ktpool", bufs=4) as ktp,
            tc.tile_pool(name="atpool", bufs=1) as atp,
            tc.tile_pool(name="aoacc", bufs=1) as aop,
            tc.tile_pool(name="w2s", bufs=10) as w2s,
        ):
            qT_sb = st.tile([128, QPC], F16, tag="qT")
            mask_sb = st.tile([128, 16], F32, tag="mask")
            recip_sb = st.tile([128, QPC], F32, tag="recip")
            # full-width ones: the denominator matmul then lands broadcast
            # across all 128 PSUM partitions (the PE array is 128 wide
            # either way), so no partition_broadcast is needed afterwards.
            ones_sb = st.tile([128, 128], BF16, tag="ones")
            nc.vector.memset(ones_sb[:], 1.0)

            aoacc = [
                aop.tile([128, QPC], BF16, tag=f"aoacc{oc}", name=f"aoacc{oc}")
                for oc in range(NDM)
            ]

            # ---- ACT exp-table pre-warm (before first real exp) ----
            warm_sb = st.tile([1, 2], F32, tag="warm")
            nc.vector.memset(warm_sb[:], 0.0)
            nc.scalar.activation(
                warm_sb[:], warm_sb[:], mybir.ActivationFunctionType.Exp
            )

            # ---- DMA issues: startup split across sync+scalar queues ----
            es1 = ExitStack()
            p1 = es1.enter_context(tc.tile_pool(name="p1", bufs=1))
            xts = es1.enter_context(tc.tile_pool(name="xts", bufs=3))

            # single sync queue, strict need order: DMA engines round-robin
            # across queues, so a second queue would let non-critical bytes
            # race ahead of the kproj(0)/qproj feed.
            # wk_ic[ic]: [128,128] lhs APs; x0_ic[ic]: [128,256] rhs APs.
            # The first (ic=0) pair gets its own small DMAs so the very
            # first matmul starts on 96KB instead of 768KB of arrivals.
            wk_ic, x0_ic, xq_t = [None] * NDM, [None] * NDM, []
            wq_sb = None
            for cq in range(4):
                if cq == 0:
                    twk_a = p1.tile([128, 128], F16, tag="wk0a", name="wk0a")
                    nc.sync.dma_start(out=twk_a[:], in_=wkr[:, 0:128])
                    tx0_a = p1.tile([128, 256], F16, tag="x00a", name="x00a")
                    nc.sync.dma_start(out=tx0_a[:], in_=xtp[0][:, 0:256])
                    twk_b = p1.tile([128, 384], F16, tag="wk0b", name="wk0b")
                    nc.sync.dma_start(out=twk_b[:], in_=wkr[:, 128:512])
                    tx0_b = p1.tile([128, 768], F16, tag="x00b", name="x00b")
                    nc.sync.dma_start(out=tx0_b[:], in_=xtp[0][:, 256:1024])
                    wk_ic[0] = twk_a[:]
                    x0_ic[0] = tx0_a[:]
                    for j in range(1, 4):
                        wk_ic[j] = twk_b[:, 128 * (j - 1) : 128 * j]
                        x0_ic[j] = tx0_b[:, 256 * (j - 1) : 256 * j]
                else:
                    twk = p1.tile([128, 512], F16, tag=f"wk{cq}", name=f"wk{cq}")
                    nc.sync.dma_start(
                        out=twk[:], in_=wkr[:, 512 * cq : 512 * (cq + 1)]
                    )
                    tx0 = p1.tile([128, 1024], F16, tag=f"x0{cq}", name=f"x0{cq}")
                    nc.sync.dma_start(
                        out=tx0[:], in_=xtp[0][:, 1024 * cq : 1024 * (cq + 1)]
                    )
                    for j in range(4):
                        wk_ic[4 * cq + j] = twk[:, 128 * j : 128 * (j + 1)]
                        x0_ic[4 * cq + j] = tx0[:, 256 * j : 256 * (j + 1)]
                if cq == 0:
                    wq_sb = p1.tile([128, D_MODEL], F16, tag="wq")
                    nc.sync.dma_start(out=wq_sb[:], in_=wqr[:])
                txq = p1.tile([128, 4 * QPC], F16, tag=f"xq{cq}", name=f"xq{cq}")
                nc.sync.dma_start(
                    out=txq[:], in_=xqr[:, 4 * QPC * cq : 4 * QPC * (cq + 1)]
                )
                xq_t.append(txq)
            nc.sync.dma_start(out=mask_sb[:], in_=maskb[:])

            # sync queue: xts stream with xv issues pulled one group earlier
            # than consumption (pool 17-deep -> no pacing waits until late).
            XV_ALLOC = (
                [[0, 1, 2], [3, 4, 5], [6, 7, 8]]
                + [[9 + 2 * j, 10 + 2 * j] for j in range(11)]
                + [[31]]
            )
            xts_t = [None]
            xv_t = {}
            for i in range(1, NKG):
                t = xts.tile([128, NDM * KG], F16, tag="xts", name=f"xts{i}")
                nc.sync.dma_start(out=t[:], in_=xtp[i])
                xts_t.append(t)
                for kt in XV_ALLOC[i - 1]:
                    tv = xvp.tile([128, D_MODEL], F16, tag="xv", name=f"xv{kt}")
                    nc.sync.dma_start(
                        out=tv[:], in_=xv[128 * kt : 128 * (kt + 1), :]
                    )
                    xv_t[kt] = tv
            xv_t = [xv_t[kt] for kt in range(NKT)]
            tw_t = {}
            for oc in range(10):
                tw = w2s.tile([128, D_MODEL], BF16, tag="w2", name=f"w2_{oc}")
                nc.sync.dma_start(out=tw[:], in_=w2r[oc])
                tw_t[oc] = tw

            at_t = []
            ktile_t = {}

            def kproj(i):
                psk = pskp.tile([128, KG], F32, tag="psk", name=f"psk{i}")
                for ic in range(NDM):
                    lhs = wk_ic[ic]
                    rhs = xts_t[i][:, KG * ic : KG * (ic + 1)]
                    nc.tensor.matmul(
                        psk[:], lhs, rhs, start=(ic == 0), stop=(ic == NDM - 1)
                    )
                ktile = ktp.tile([128, KG], F16, tag="kt", name=f"kt{i}")
                nc.vector.tensor_copy(ktile[:], psk[:])
                ktile_t[i] = ktile

            def scores(kt):
                w = W[kt]
                ps = pssp.tile([128, 512], F32, tag="pss", name=f"pss{kt}")
                nc.tensor.matmul(
                    ps[:, :w],
                    ktile_t[kt // 2][:, 128 * (kt % 2) : 128 * (kt % 2 + 1)],
                    qT_sb[:, QPC - w : QPC],
                    start=True, stop=True,
                )
                nc.vector.tensor_add(ps[:, :16], ps[:, :16], mask_sb[:])
                at = atp.tile([128, w], BF16, tag=f"at{kt}")
                nc.scalar.activation(
                    at[:], ps[:, :w], mybir.ActivationFunctionType.Exp
                )
                at_t.append(at)

            def denom(kt):
                w = W[kt]
                nc.tensor.matmul(
                    psd[:, QPC - w : QPC], ones_sb[:], at_t[kt][:],
                    start=(kt == 0), stop=(kt == NKT - 1),
                )

            def v_stream(k0, klen, oc):
                w0 = W[k0]
                vt = vpp.tile([128, QPC], F32, tag="vps", name=f"v{k0}_{oc}")
                for kt in range(k0, k0 + klen):
                    w = W[kt]
                    nc.tensor.matmul(
                        vt[:, QPC - w : QPC],
                        xv_t[kt][:, 128 * oc : 128 * (oc + 1)],
                        at_t[kt][:],
                        start=(kt == k0), stop=(kt == k0 + klen - 1),
                    )
                if k0 == 0:
                    nc.vector.tensor_copy(aoacc[oc][:], vt[:])
                else:
                    nc.vector.tensor_add(
                        aoacc[oc][:, QPC - w0 : QPC],
                        aoacc[oc][:, QPC - w0 : QPC],
                        vt[:, QPC - w0 : QPC],
                    )

            es2 = ExitStack()
            pskp = es2.enter_context(tc.tile_pool(name="psk", bufs=2, space="PSUM"))
            pssp = es2.enter_context(tc.tile_pool(name="pss", bufs=2, space="PSUM"))
            psdp = es2.enter_context(tc.tile_pool(name="psd", bufs=1, space="PSUM"))
            psd = psdp.tile([128, QPC], F32, tag="psd")

            # ---- startup: kproj(0) and qproj chunk-interleaved ----
            with tc.tile_pool(name="psq", bufs=1, space="PSUM") as psqp:
                psk = pskp.tile([128, KG], F32, tag="psk", name="psk0")
                psq = psqp.tile([128, QPC], F32, tag="psq")
                for cq in range(4):
                    for j in range(4):
                        ic = 4 * cq + j
                        nc.tensor.matmul(
                            psk[:],
                            wk_ic[ic],
                            x0_ic[ic],
                            start=(ic == 0), stop=(ic == NDM - 1),
                        )
                    for j in range(4):
                        ic = 4 * cq + j
                        nc.tensor.matmul(
                            psq[:],
                            wq_sb[:, 128 * ic : 128 * (ic + 1)],
                            xq_t[cq][:, QPC * j : QPC * (j + 1)],
                            start=(ic == 0), stop=(ic == NDM - 1),
                        )
                ktile = ktp.tile([128, KG], F16, tag="kt", name="kt0")
                nc.vector.tensor_copy(ktile[:], psk[:])
                ktile_t[0] = ktile
                nc.vector.tensor_copy(qT_sb[:], psq[:])
            vpp = es2.enter_context(
                tc.tile_pool(name="vps", bufs=3, space="PSUM", side="right")
            )

            # ---- pipelined main loop ----
            for i in range(1, NKG):
                # Interleave the 4 V streams with kproj/scores so the DVE
                # merges catch up between streams (back-to-back streams stall
                # on vps bank reuse).  Not possible when s==0: that group's
                # at(2i-2),at(2i-1) are produced by THIS iteration's scores,
                # and the in-order PE queue would deadlock.
                g = i // 4 - 1
                s = i % 4
                vocs = list(range(4 * s, 4 * s + 4)) if i >= 4 else []
                if vocs and s != 0:
                    kproj(i)
                    v_stream(GS * g, GS, vocs[0])
                    scores(2 * i - 2)
                    v_stream(GS * g, GS, vocs[1])
                    scores(2 * i - 1)
                    v_stream(GS * g, GS, vocs[2])
                    denom(2 * i - 6)
                    denom(2 * i - 5)
                    v_stream(GS * g, GS, vocs[3])
                else:
                    kproj(i)
                    scores(2 * i - 2)
                    scores(2 * i - 1)
                    if i >= 3:
                        denom(2 * i - 6)
                        denom(2 * i - 5)
                    for oc in vocs:
                        v_stream(GS * g, GS, oc)
                if i == NKG - 1:
                    scores(NKT - 2)
                    scores(NKT - 1)
                    denom(NKT - 6)
                    denom(NKT - 5)
            denom(NKT - 4)
            denom(NKT - 3)
            denom(NKT - 2)
            denom(NKT - 1)
            # reciprocal in four [128,128] chunks interleaved between the
            # final v_streams: one long DVE reciprocal would otherwise block
            # the aoacc merges feeding the first W2 chain.
            for oc in range(NDM):
                v_stream(24, 8, oc)
                if oc in (0, 2, 4, 6):
                    c = oc // 2
                    nc.vector.reciprocal(
                        recip_sb[:, 128 * c : 128 * (c + 1)],
                        psd[:, 128 * c : 128 * (c + 1)],
                    )
            es2.close()
            es1.close()

            # ---- W2 ----
            with (
                tc.tile_pool(name="outs", bufs=2) as outs,
                tc.tile_pool(name="ps4", bufs=4, space="PSUM") as ps4,
            ):
                pair = None
                for oc in range(NDM - 1):
                    if oc in tw_t:
                        tw = tw_t[oc]
                    else:
                        tw = w2s.tile([128, D_MODEL], BF16, tag="w2",
                                      name=f"w2_{oc}")
                        nc.sync.dma_start(out=tw[:], in_=w2r[oc])
                    ps = ps4.tile([128, QPC], F32, tag="ps4")
                    for ic in range(NDM):
                        nc.tensor.matmul(
                            ps[:],
                            tw[:, 128 * ic : 128 * (ic + 1)],
                            aoacc[ic][:],
                            start=(ic == 0), stop=(ic == NDM - 1),
                        )
                    if oc < NDM - 2:
                        if oc % 2 == 0:
                            pair = outs.tile([128, 2 * QPC], F16, tag="out")
                        half = oc % 2
                        nc.vector.tensor_mul(
                            pair[:, QPC * half : QPC * (half + 1)], ps[:],
                            recip_sb[:],
                        )
                        if oc % 2 == 1:
                            nc.scalar.dma_start(
                                out=outT2[:, QPC * (oc - 1) : QPC * (oc + 1)],
                                in_=pair[:],
                            )
                    else:
                        sng = outs.tile([128, QPC], F16, tag="sng", name=f"sng{oc}")
                        nc.vector.tensor_mul(sng[:], ps[:], recip_sb[:])
                        nc.scalar.dma_start(
                            out=outT2[:, QPC * oc : QPC * (oc + 1)], in_=sng[:]
                        )
                # last oc as two half-width chains: the first half's
                # normalize+store overlaps the second half's matmuls.
                oc = NDM - 1
                tw = w2s.tile([128, D_MODEL], BF16, tag="w2", name=f"w2_{oc}")
                nc.sync.dma_start(out=tw[:], in_=w2r[oc])
                H = QPC // 2
                for half in range(2):
                    ps = ps4.tile([128, QPC], F32, tag="ps4")
                    for ic in range(NDM):
                        nc.tensor.matmul(
                            ps[:, :H],
                            tw[:, 128 * ic : 128 * (ic + 1)],
                            aoacc[ic][:, H * half : H * (half + 1)],
                            start=(ic == 0), stop=(ic == NDM - 1),
                        )
                    sng = outs.tile(
                        [128, H], F16, tag="sngh", name=f"sng{oc}_{half}"
                    )
                    nc.vector.tensor_mul(
                        sng[:], ps[:, :H], recip_sb[:, H * half : H * (half + 1)]
                    )
                    if half == 0:
                        nc.scalar.dma_start(
                            out=outT2[:, QPC * oc : QPC * oc + H], in_=sng[:]
                        )
                    else:
                        nc.sync.dma_start(
                            out=outT2[:, QPC * oc + H : QPC * (oc + 1)], in_=sng[:]
                        )

    nc.compile()
    return nc


def prepare_inputs(x, Wk, Wq, W2):
    x = np.asarray(x, dtype=np.float32)
    Wk = np.asarray(Wk, dtype=np.float32)
    Wq = np.asarray(Wq, dtype=np.float32)
    W2 = np.asarray(W2, dtype=np.float32)

    xT16 = np.ascontiguousarray(x.T).astype(np.float16)
    xtp = np.ascontiguousarray(
        xT16.reshape(NDM, 128, NKG, KG).transpose(2, 1, 0, 3).reshape(NKG, 128, NDM * KG)
    )
    xv16 = x.astype(np.float16)

    def pack_chunks(aT, width):
        return np.ascontiguousarray(
            aT.reshape(NDM, 128, width).transpose(1, 0, 2).reshape(128, NDM * width)
        )

    wqr = pack_chunks(np.ascontiguousarray(Wq.T).astype(np.float16), D_HEAD)
    wkr = pack_chunks(np.ascontiguousarray(Wk.T).astype(np.float16), D_HEAD)
    w2T = np.ascontiguousarray(W2.T).astype(ml_dtypes.bfloat16)
    w2r = np.ascontiguousarray(
        w2T.reshape(NDM, 128, NDM, 128).transpose(2, 1, 0, 3).reshape(NDM, 128, D_MODEL)
    )

    in_maps = []
    for c in range(NCORES):
        xqT = np.ascontiguousarray(x[c::NCORES].T).astype(np.float16)
        xqr_c = pack_chunks(xqT, QPC)
        mask = np.zeros((128, 16), dtype=np.float32)
        j = np.arange(128)[:, None]
        t = np.arange(16)[None, :]
        mask[j > 8 * t + c] = MASK_NEG
        in_maps.append(
            {
                "xqr": xqr_c,
                "xtp": xtp,
                "xv": xv16,
                "wqr": wqr,
                "wkr": wkr,
                "w2r": w2r,
                "maskb": mask,
            }
        )
    return in_maps


def assemble_output(results):
    res = np.stack(
        [np.asarray(results[c]["outT2"]).astype(np.float32) for c in range(NCORES)]
    )
    res = res.reshape(NCORES, 128, NDM, QPC)
    return np.ascontiguousarray(
        res.transpose(3, 0, 2, 1).reshape(N_CTX, D_MODEL)
    )


_CACHED = {}


def kernel(x, Wk, Wq, W2, _trace=False):
    if "nc" not in _CACHED:
        _CACHED["nc"] = build_program()
    nc = _CACHED["nc"]
    in_maps = prepare_inputs(x, Wk, Wq, W2)
    res = run_bass_kernel_spmd(nc, in_maps, core_ids=list(range(NCORES)), trace=_trace)
    out = assemble_output(res.results)
    if _trace:
        return out, res
    return out

